# revision 37
# baseline (speedup 1.0000x reference)
"""CrossCityAdapter Trainium2 kernel (8 NeuronCores, SPMD).

Math (per batch b):
  s_ad = relu(LN(src @ sW1 + sb1)) @ sW2 + sb2          [S,H]
  t_ad = relu(LN(tgt @ tW1 + tb1)) @ tW2 + tb2          [T,H]
  pre[t,s,:] = t_ad@cW_t[t,:] + (s_ad@cW_s + cb)[s,:]   (separable over t/s)
  h = relu(LN(pre)*cg + cbeta);  scores = sigmoid(h @ simW + simb)
  attn = softmax(scores, -1); transferred = attn @ s_ad
  gate = sigmoid(mean(t_ad)); adapted = t_ad*(1-gate) + transferred*gate

Key identities exploited on-device:
  * LN stats of pre decompose: mean = mt[t]+ms[s],
    var = a[t] + c[s] + (2/H) * (xhat @ vhat^T)[t,s]  -> one PE matmul
    (a/c/1-rows appended to the operands so one matmul yields full var).
  * With cbeta*|simW| == 0 (true for these inputs) and w = rstd > 0:
      sum_h simW[h]*relu((xhat+vhat)*w*cg + cbeta)
        = w * (sum_h sgn[h]*max(xtil,-vtil) + K[s]),   xtil = xhat*cg*|simW|
    using relu(a+b) = max(a,-b)+b and K[s] = sum_h vhat[s,h]*cg[h]*simW[h].
    So the only O(T*S*H) elementwise op is one bf16 max on DVE. The signed
    H-reduction runs on the tensor engine: the max-result r (h-stacked for
    two targets, K=128 = 2x64) is the STATIONARY operand and the +-1 sign
    matrix streams, so PSUM gets [128 s, 2] full-partition outputs (exact:
    bf16 * {+-1,0} accumulated in fp32).
  * Everything downstream of the sign-reduce is s-major; scores are
    PE-transposed back to t-major only for the final DMA, and the s-major
    exp(scores) feeds the attn matmul as lhsT with no transpose at all.
  * sigmoid outputs lie in (0,1) so softmax needs no max-subtraction.

Sharding: 8 cores = batch (2) x T-quarters (4); each core owns 256 targets,
all S. No collectives; host gathers core outputs.

Target pairing: column c in [0,128) pairs local targets (c, c+128);
"slot" 2c+j (j = half) <-> t = c + 128j. PSUM/num rows use slot order;
DRAM APs unscramble on the way out.
"""

import numpy as np

B, S, T, DS, DT, H = 2, 1024, 1024, 128, 64, 64
LN_EPS = 1e-5
NCORES = 8
TLOC = T * B // NCORES  # 256 target rows per core

_CACHE = {}


def _build_program(wmeta):
    import concourse.bacc as bacc
    import concourse.bass as bass
    import concourse.tile as tile
    from concourse import mybir
    from concourse.masks import make_identity

    f32 = mybir.dt.float32
    bf16 = mybir.dt.bfloat16
    AF = mybir.ActivationFunctionType
    OP = mybir.AluOpType
    AX = mybir.AxisListType

    simb = wmeta["simb"]
    triv = wmeta["trivial"]

    nc = bacc.Bacc("TRN2", target_bir_lowering=False, debug=False)

    # ---- DRAM I/O ------------------------------------------------------
    src = nc.declare_dram_parameter("src", [S, DS], f32, isOutput=False).ap()
    tgt = nc.declare_dram_parameter("tgt", [TLOC, DT], f32, isOutput=False).ap()
    sW1 = nc.declare_dram_parameter("sW1", [DS, H], f32, isOutput=False).ap()
    sW2 = nc.declare_dram_parameter("sW2", [H, H], f32, isOutput=False).ap()
    tW1 = nc.declare_dram_parameter("tW1", [DT, H], f32, isOutput=False).ap()
    tW2 = nc.declare_dram_parameter("tW2", [H, H], f32, isOutput=False).ap()
    cWt = nc.declare_dram_parameter("cWt", [H, H], f32, isOutput=False).ap()
    cWs = nc.declare_dram_parameter("cWs", [H, H], f32, isOutput=False).ap()
    gvec2 = nc.declare_dram_parameter("gvec2", [128], f32, isOutput=False).ap()
    sgn2 = nc.declare_dram_parameter("sgn2", [128, 2], f32, isOutput=False).ap()
    sgng = nc.declare_dram_parameter("sgng", [H], f32, isOutput=False).ap()
    nontriv_vecs = {}
    for name in ("sb1", "sg", "sbeta", "sb2", "tb1", "tg", "tbeta", "tb2", "cb"):
        if not triv[name]:
            nontriv_vecs[name] = nc.declare_dram_parameter(
                name, [H], f32, isOutput=False
            ).ap()
    scores_out = nc.declare_dram_parameter(
        "scores_out", [TLOC, S], f32, isOutput=True
    ).ap()
    adapted_out = nc.declare_dram_parameter(
        "adapted_out", [TLOC, H], f32, isOutput=True
    ).ap()

    def bcast_row(ap, n):
        return bass.AP(tensor=ap.tensor, offset=ap.offset,
                       ap=[[0, 128], [1, n]])

    _ps_pools = []
    _ps_ctr = [0]
    with tile.TileContext(nc) as tc:
        with (
            tc.tile_pool(name="singles", bufs=1) as singles,
            tc.tile_pool(name="weights", bufs=1) as wpool,
            tc.tile_pool(name="temps", bufs=4) as temps,
            tc.tile_pool(name="cols", bufs=4) as cols,
            tc.tile_pool(name="rbig", bufs=4) as rbig_pool,
            tc.tile_pool(name="smaj", bufs=4) as smaj,
            # PSUM budget (8 banks): psT 2 + psV 2 + psSC 2 + psN 2 = 8
            tc.tile_pool(name="psT", bufs=2, space="PSUM") as psT,
            tc.tile_pool(name="psV", bufs=2, space="PSUM") as psV,
            tc.tile_pool(name="psSC", bufs=2, space="PSUM") as psSC,
            tc.tile_pool(name="psN", bufs=2, space="PSUM") as psN,
            tc.tile_pool(name="dram", bufs=1, space="DRAM") as dram,
        ):
            _ps_pools.extend([(psT, "ps")])
            def ps128():
                """Round-robin a [128,128] psum tile across all pools
                (psSC/psN are idle during the adapter phase)."""
                pool, tag = _ps_pools[_ps_ctr[0] % len(_ps_pools)]
                _ps_ctr[0] += 1
                return pool.tile([128, 128], f32, tag=tag, name="ps128")

            # ---- constants / weights in SBUF --------------------------
            ident = singles.tile([128, 128], f32)
            make_identity(nc, ident[:])

            sW1_sb = wpool.tile([DS, H], f32)
            nc.sync.dma_start(out=sW1_sb[:], in_=sW1)
            w64 = {}
            for name, ap in (("sW2", sW2), ("tW1", tW1), ("tW2", tW2),
                             ("cWt", cWt), ("cWs", cWs)):
                t_ = wpool.tile([H, H], f32, tag=name, name=f"w_{name}")
                nc.sync.dma_start(out=t_[:], in_=ap)
                w64[name] = t_

            gvec2_col = wpool.tile([128, 1], f32)
            nc.sync.dma_start(out=gvec2_col[:], in_=gvec2[:, None])

            sgn2_f = wpool.tile([128, 2], f32)
            nc.sync.dma_start(out=sgn2_f[:], in_=sgn2)
            sgn2_bf = wpool.tile([128, 2], bf16)
            nc.vector.tensor_copy(sgn2_bf[:], sgn2_f[:])

            sgng_rep = wpool.tile([128, H], f32)
            nc.sync.dma_start(out=sgng_rep[:], in_=bcast_row(sgng, H))

            eps_col = wpool.tile([128, 1], f32)
            nc.vector.memset(eps_col[:], LN_EPS)

            vrep = {}
            for name, ap in nontriv_vecs.items():
                t_ = wpool.tile([128, H], f32, tag=f"{name}_rep",
                                name=f"rep_{name}")
                nc.sync.dma_start(out=t_[:], in_=bcast_row(ap, H))
                vrep[name] = t_

            # ---- persistent device tensors ----------------------------
            # (split by s-block so the main loop overlaps the adapters)
            xstk = singles.tile([128, 128], f32)      # rows h|h: xtil^T t|t+128
            vstk = [singles.tile([128, 512], bf16, name=f"vstk{z}")
                    for z in range(2)]                # rows h|h: -vtil^T (dup)
            xhatT_ext = singles.tile([66, TLOC], f32)  # [xhat^T; ones; a+eps]
            vext = [singles.tile([66, 128], f32, name=f"vext{u}")
                    for u in range(S // 128)]          # [vhat^T*2/H; c; 1]
            sad_t = [singles.tile([128, H + 1], f32, name=f"sad{u}")
                     for u in range(S // 128)]
            t_ad_sb = singles.tile([128, 2, H], f32)
            kcol_t = [singles.tile([128, 1], f32, name=f"kcol{u}")
                      for u in range(S // 128)]        # K[s], s-major

            a_dram = dram.tile([TLOC], f32)
            c_dram = [dram.tile([128], f32, name=f"c_dram{u}")
                      for u in range(S // 128)]
            tad_dram = dram.tile([TLOC, H], f32)
            gate_dram = dram.tile([TLOC], f32)

            nc.vector.memset(xhatT_ext[64:66, :], 1.0)


            def ln_relu(a_ps, bias_rep, g_rep, beta_rep):
                """psum [128,H] -> relu(LN(.)) in sbuf."""
                a_sb_t = temps.tile([128, H], f32, tag="ln_in",
                                    name="ln_in")
                if bias_rep is not None:
                    nc.vector.tensor_add(a_sb_t[:], a_ps, bias_rep[:])
                else:
                    nc.scalar.copy(a_sb_t[:], a_ps)
                a_sb = a_sb_t[:]
                st6 = temps.tile([128, 6], f32, tag="ln_st", name="ln_st")
                mv = temps.tile([128, 2], f32, tag="ln_mv", name="ln_mv")
                nc.vector.bn_stats(out=st6[:], in_=a_sb)
                nc.vector.bn_aggr(out=mv[:], in_=st6[:])
                std = temps.tile([128, 1], f32, tag="ln_std", name="ln_std")
                nc.scalar.activation(std[:], mv[:, 1:2], AF.Sqrt,
                                     bias=eps_col[:], scale=1.0)
                rstd = temps.tile([128, 1], f32, tag="ln_rstd", name="ln_rstd")
                nc.vector.reciprocal(rstd[:], std[:])
                xn = temps.tile([128, H], f32, tag="ln_xn", name="ln_xn")
                nc.vector.tensor_scalar(
                    out=xn[:], in0=a_sb, scalar1=mv[:, 0:1],
                    scalar2=rstd[:], op0=OP.subtract, op1=OP.mult)
                if g_rep is not None:
                    nc.vector.tensor_mul(xn[:], xn[:], g_rep[:])
                if beta_rep is not None:
                    nc.vector.tensor_add(xn[:], xn[:], beta_rep[:])
                out = temps.tile([128, H], f32, tag="ln_out", name="ln_out")
                nc.scalar.activation(out[:], xn[:], AF.Relu)
                return out

            def transpose_small(in_ap, tag="tp"):
                """[128,F<=128] sbuf -> [F,128] sbuf via PE transpose."""
                ps = ps128()
                fsz = in_ap.shape[-1]
                nc.tensor.transpose(ps[0:fsz, :], in_ap, ident[:])
                sb = temps.tile([128, 128], f32, tag=tag, name=f"sb_{tag}")
                nc.scalar.copy(sb[0:fsz, :], ps[0:fsz, :])
                return sb

            # ---- target adapter (256 rows) ----------------------------
            gate_ps = []
            xstk_stage = temps.tile([64, 128], f32, tag="xstg",
                                    name="xstk_stage")
            for j in range(2):
                x_sb = temps.tile([128, DT], f32, tag="tgt_in", name="tgt_in")
                nc.sync.dma_start(out=x_sb[:], in_=tgt[j * 128:(j + 1) * 128, :])
                xT_sb = transpose_small(x_sb[:], tag="tgtT")
                a1_ps = ps128()
                nc.tensor.matmul(a1_ps[:, 0:H], xT_sb[0:DT, :], w64["tW1"][:])
                a1r = ln_relu(a1_ps[:, 0:H], vrep.get("tb1"), vrep.get("tg"),
                              vrep.get("tbeta"))
                a1rT = transpose_small(a1r[:], tag="a1rT")
                tad_ps = ps128()
                nc.tensor.matmul(tad_ps[:, 0:H], a1rT[0:H, :], w64["tW2"][:])
                if "tb2" in vrep:
                    nc.vector.tensor_add(t_ad_sb[:, j, :], tad_ps[:, 0:H],
                                         vrep["tb2"][:])
                else:
                    nc.scalar.copy(t_ad_sb[:, j, :], tad_ps[:, 0:H])
                nc.sync.dma_start(out=tad_dram[j * 128:(j + 1) * 128, :],
                                  in_=t_ad_sb[:, j, :])
                gsum = cols.tile([128, 1], f32, tag=f"gps{j}", name=f"gps{j}")
                nc.vector.tensor_reduce(out=gsum[:], in_=t_ad_sb[:, j, :],
                                        axis=AX.X, op=OP.add)
                nc.sync.dma_start(out=gate_dram[j * 128:(j + 1) * 128],
                                  in_=gsum[:])
                gate_ps.append(gsum)

                tadT = transpose_small(t_ad_sb[:, j, :], tag="tadT")
                xp_ps = ps128()
                nc.tensor.matmul(xp_ps[:, 0:H], tadT[0:H, :], w64["cWt"][:])
                x_pre_t = temps.tile([128, H], f32, tag="x_pre",
                                     name="x_pre")
                nc.scalar.copy(x_pre_t[:], xp_ps[:, 0:H])
                x_pre = x_pre_t[:]
                st6 = temps.tile([128, 6], f32, tag="ln_st", name="st6t")
                mv = temps.tile([128, 2], f32, tag="ln_mv", name="mvt")
                nc.vector.bn_stats(out=st6[:], in_=x_pre)
                nc.vector.bn_aggr(out=mv[:], in_=st6[:])
                a_eps = temps.tile([128, 1], f32, tag="a_eps", name="a_eps")
                nc.vector.tensor_scalar(out=a_eps[:], in0=mv[:, 1:2],
                                        scalar1=LN_EPS, scalar2=None,
                                        op0=OP.add)
                nc.sync.dma_start(out=a_dram[j * 128:(j + 1) * 128],
                                  in_=a_eps[:])
                xhat = temps.tile([128, H], f32, tag="xhat", name="xhat")
                nc.vector.tensor_scalar(out=xhat[:], in0=x_pre,
                                        scalar1=mv[:, 0:1], scalar2=None,
                                        op0=OP.subtract)
                xf_ps = ps128()
                nc.tensor.transpose(xf_ps[0:H, :], xhat[:], ident[:])
                nc.scalar.copy(xhatT_ext[0:H, j * 128:(j + 1) * 128],
                               xf_ps[0:H, :])
                if j == 0:
                    nc.vector.tensor_scalar(
                        out=xstk[0:H, :], in0=xf_ps[0:H, :],
                        scalar1=gvec2_col[0:H, :], scalar2=None, op0=OP.mult)
                else:
                    nc.vector.tensor_scalar(
                        out=xstk_stage[:], in0=xf_ps[0:H, :],
                        scalar1=gvec2_col[0:H, :], scalar2=None, op0=OP.mult)
                    nc.sync.dma_start(out=xstk[H:128, :], in_=xstk_stage[:])
            nc.sync.dma_start(out=xhatT_ext[64:65, :],
                              in_=bass.AP(tensor=a_dram[:].tensor,
                                          offset=a_dram[:].offset,
                                          ap=[[0, 1], [1, TLOC]]))

            # ---- source adapter + v-side (full S) ---------------------
            for i in range(S // 128):
                x_sb = temps.tile([128, DS], f32, tag="src_in", name="src_in")
                nc.sync.dma_start(out=x_sb[:], in_=src[i * 128:(i + 1) * 128, :])
                xT_sb = transpose_small(x_sb[:], tag="srcT")
                a1_ps = ps128()
                nc.tensor.matmul(a1_ps[:, 0:H], xT_sb[:], sW1_sb[:])
                a1r = ln_relu(a1_ps[:, 0:H], vrep.get("sb1"), vrep.get("sg"),
                              vrep.get("sbeta"))
                a1rT = transpose_small(a1r[:], tag="a1rT")
                sad_ps = ps128()
                nc.tensor.matmul(sad_ps[:, 0:H], a1rT[0:H, :], w64["sW2"][:])
                if "sb2" in vrep:
                    nc.vector.tensor_add(sad_t[i][:, 0:H], sad_ps[:, 0:H],
                                         vrep["sb2"][:])
                else:
                    nc.scalar.copy(sad_t[i][:, 0:H], sad_ps[:, 0:H])
                nc.vector.memset(sad_t[i][:, H:H + 1], 1.0)

                sadT = transpose_small(sad_t[i][:, 0:H], tag="sadT")
                v_ps = ps128()
                nc.tensor.matmul(v_ps[:, 0:H], sadT[0:H, :], w64["cWs"][:])
                v_pre_t = temps.tile([128, H], f32, tag="v_pre",
                                     name="v_pre")
                if "cb" in vrep:
                    nc.vector.tensor_add(v_pre_t[:], v_ps[:, 0:H],
                                         vrep["cb"][:])
                else:
                    nc.scalar.copy(v_pre_t[:], v_ps[:, 0:H])
                v_pre = v_pre_t[:]
                st6 = temps.tile([128, 6], f32, tag="ln_st", name="st6v")
                mv = temps.tile([128, 2], f32, tag="ln_mv", name="mvv")
                nc.vector.bn_stats(out=st6[:], in_=v_pre)
                nc.vector.bn_aggr(out=mv[:], in_=st6[:])
                vhat = temps.tile([128, H], f32, tag="vhat", name="vhat")
                nc.vector.tensor_scalar(out=vhat[:], in0=v_pre,
                                        scalar1=mv[:, 0:1], scalar2=None,
                                        op0=OP.subtract)
                # K[s] = sum_h vhat*cg*simW  (stays per-partition, s-major)
                kv = temps.tile([128, H], f32, tag="kv", name="kv")
                nc.vector.tensor_mul(kv[:], vhat[:], sgng_rep[:])
                nc.vector.tensor_reduce(out=kcol_t[i][:], in_=kv[:],
                                        axis=AX.X, op=OP.add)
                vT_ps = ps128()
                nc.tensor.transpose(vT_ps[0:H, :], vhat[:], ident[:])
                nc.vector.tensor_scalar(
                    out=vext[i][0:H, :],
                    in0=vT_ps[0:H, :], scalar1=2.0 / H, scalar2=None,
                    op0=OP.mult)
                zz, zo = i // 4, (i % 4) * 128
                nc.vector.tensor_scalar(
                    out=vstk[zz][0:H, zo:zo + 128], in0=vT_ps[0:H, :],
                    scalar1=gvec2_col[0:H, :], scalar2=-1.0,
                    op0=OP.mult, op1=OP.mult)
                nc.sync.dma_start(out=vstk[zz][H:128, zo:zo + 128],
                                  in_=vstk[zz][0:H, zo:zo + 128])
                nc.sync.dma_start(out=c_dram[i][:], in_=mv[:, 1:2])
                # rows 64:66 <- [ones; c]: broadcast c into both (base-64
                # DMA), then memset row 64 back to 1.0 (base-64 DVE op).
                nc.sync.dma_start(
                    out=vext[i][64:66, :],
                    in_=bass.AP(tensor=c_dram[i][:].tensor,
                                offset=c_dram[i][:].offset,
                                ap=[[0, 2], [1, 128]]))
                nc.vector.memset(vext[i][64:65, :], 1.0)

            # funny-gathered epilogue inputs per half f:
            #   row r <-> t = 64*f + r//2 + 128*(r&1)
            tad_re = tad_dram[:].rearrange("(j c) h -> c j h", j=2)
            gate_re = gate_dram[:].rearrange("(j c) -> c j", j=2)
            tadf, gatef = [], []
            for f in range(2):
                tf = cols.tile([128, H], f32, tag="tadf", name=f"tadf{f}")
                nc.sync.dma_start(out=tf[:],
                                  in_=tad_re[64 * f:64 * (f + 1), :, :])
                tadf.append(tf)
                gf = cols.tile([128, 1], f32, tag="gatef", name=f"gatef{f}")
                nc.sync.dma_start(out=gf[:],
                                  in_=gate_re[64 * f:64 * (f + 1), :])
                gatef.append(gf)

            # ---- main pipeline, phase-ordered to batch ACT table sets --
            # xhatT funny view: col (2c+j) <-> t = c + 128j
            xfun = xhatT_ext[:].rearrange("p (j c) -> p c j", j=2)
            scout_re = scores_out.rearrange("(j c) s -> c j s", j=2)
            num_ps = [psN.tile([128, H + 1], f32, tag="num", name=f"num{f}")
                      for f in range(2)]
            NB = S // 128  # 8 s-blocks

            # W-phase: w = rsqrt(var) per s-block (Sqrt set stays loaded)
            w_t = []
            sqrt_insts = []
            for i_s in range(NB):
                var_ps = psT.tile([128, 256], f32, tag="ps", name="var_ps")
                nc.tensor.matmul(var_ps[:], vext[i_s][:], xfun)
                std = temps.tile([128, 256], f32, tag="std", name="std")
                sqrt_insts.append(
                    nc.scalar.activation(std[:], var_ps[:], AF.Sqrt))
                w_ = smaj.tile([128, 256], f32, tag="w", name=f"w{i_s}",
                               bufs=8)
                nc.vector.reciprocal(w_[:], std[:])
                w_t.append(w_)

            # R-phase: bf16 max (tensor_scalar 4x) + PE sign-reduce + STT
            scorep = [None] * NB
            for sh in range(4):  # half of an s-512 chunk: 2 s-blocks
                sc, uh = sh // 2, sh % 2
                ps_u = [psSC.tile([128, 256], f32, tag="sc", name=f"ps_sc{u}")
                        for u in range(2)]
                for cg in range(16):
                    r_ = rbig_pool.tile([128, 8, 256], bf16, tag="rbig",
                                        name="rbig")
                    for cl in range(8):
                        nc.vector.tensor_scalar_max(
                            r_[:, cl, :],
                            vstk[sc][:, 256 * uh:256 * (uh + 1)],
                            xstk[:, 8 * cg + cl:8 * cg + cl + 1])
                    for u in range(2):
                        for cl in range(8):
                            c = 8 * cg + cl
                            nc.tensor.matmul(
                                ps_u[u][:, 2 * c:2 * c + 2],
                                r_[:, cl, 128 * u:128 * (u + 1)],
                                sgn2_bf[:], start=True, stop=True)
                for u in range(2):
                    i_s = 2 * sh + u
                    sp = smaj.tile([128, 256], f32, tag="scorep",
                                   name=f"scorep{i_s}", bufs=8)
                    nc.vector.scalar_tensor_tensor(
                        out=sp[:], in0=ps_u[u][:],
                        scalar=kcol_t[i_s][:], in1=w_t[i_s][:],
                        op0=OP.add, op1=OP.mult)
                    scorep[i_s] = sp

            # SIG-phase (one sigmoid table load)
            from concourse.tile import add_dep_helper
            gsig_t = []
            sig_insts = []
            for f in range(2):
                g_ = cols.tile([128, 1], f32, tag="gsig", name=f"gsig{f}")
                gm_ = cols.tile([128, 1], f32, tag="gm", name=f"gm{f}")
                nc.vector.tensor_scalar(out=gm_[:], in0=gatef[f][:],
                                        scalar1=1.0 / H, scalar2=None,
                                        op0=OP.mult)
                si = nc.scalar.activation(g_[:], gm_[:], AF.Sigmoid)
                add_dep_helper(si.ins, sqrt_insts[-1].ins, sync=False,
                               reason="batch ACT table sets")
                sig_insts.append(si)
                gsig_t.append(g_)
            scores_t = []
            for i_s in range(NB):
                ssb = smaj.tile([128, 256], f32, tag="scores",
                                name=f"ssb{i_s}", bufs=8)
                si = nc.scalar.activation(ssb[:], scorep[i_s][:], AF.Sigmoid,
                                          bias=float(simb), scale=1.0)
                add_dep_helper(si.ins, sqrt_insts[-1].ins, sync=False,
                               reason="batch ACT table sets")
                sig_insts.append(si)
                scores_t.append(ssb)
            exp_deps = {}
            # transpose halves to t-major, stage, and DMA in 2 big bursts
            sc_stage = [singles.tile([128, S], f32, name=f"sc_stage{f}")
                        for f in range(2)]
            for i_s in range(NB):
                for f in range(2):
                    sT_ps = psT.tile([128, 128], f32, tag="ps",
                                     name="sT_ps")
                    nc.tensor.transpose(sT_ps[:],
                                        scores_t[i_s][:, 128 * f:128 * (f + 1)],
                                        ident[:])
                    nc.vector.tensor_copy(
                        sc_stage[f][:, 128 * i_s:128 * (i_s + 1)], sT_ps[:])
            for f in range(2):
                nc.sync.dma_start(out=scout_re[64 * f:64 * (f + 1), :, :],
                                in_=sc_stage[f][:])

            # EXP-phase (one exp table load) + attn matmuls
            for i_s in range(NB):
                e_ = smaj.tile([128, 256], f32, tag="E", name=f"e{i_s}",
                               bufs=4)
                ei = nc.scalar.activation(e_[:], scores_t[i_s][:], AF.Exp)
                dep = sig_insts[2 + 5] if i_s < 6 else sig_insts[-1]
                add_dep_helper(ei.ins, dep.ins, sync=False,
                               reason="batch ACT table sets")
                if i_s == 5:
                    exp_deps["last_early"] = ei
                    # group sig6/sig7 after the early exps (4 loads total)
                    add_dep_helper(sig_insts[2 + 6].ins, ei.ins, sync=False,
                                   reason="batch ACT table sets")
                    add_dep_helper(sig_insts[2 + 7].ins, ei.ins, sync=False,
                                   reason="batch ACT table sets")
                for f in range(2):
                    nc.tensor.matmul(num_ps[f][:],
                                     e_[:, 128 * f:128 * (f + 1)],
                                     sad_t[i_s][:],
                                     start=(i_s == 0), stop=(i_s == NB - 1),
                                     skip_group_check=True)

            # ---- epilogue: adapted (slot-row order, unscrambled by DMA)
            adout_re = adapted_out.rearrange("(j c) h -> c j h", j=2)
            for f in range(2):
                zrec = cols.tile([128, 1], f32, tag="zrec", name=f"zrec{f}")
                nc.vector.reciprocal(zrec[:], num_ps[f][:, H:H + 1])
                gsig = gsig_t[f]
                trans = cols.tile([128, H], f32, tag="trans", name=f"tr{f}")
                nc.vector.tensor_scalar(out=trans[:], in0=num_ps[f][:, 0:H],
                                        scalar1=zrec[:], scalar2=None,
                                        op0=OP.mult)
                d_ = cols.tile([128, H], f32, tag="dtile", name=f"d{f}")
                nc.vector.tensor_sub(d_[:], trans[:], tadf[f][:])
                ad_ = cols.tile([128, H], f32, tag="adf", name=f"ad{f}")
                nc.vector.scalar_tensor_tensor(
                    out=ad_[:], in0=d_[:], scalar=gsig[:],
                    in1=tadf[f][:], op0=OP.mult, op1=OP.add)
                nc.sync.dma_start(out=adout_re[64 * f:64 * (f + 1), :, :],
                                  in_=ad_[:])

    nc.compile()
    return nc


def _prep(inputs):
    """Host-side weight prep. Returns (wmeta, common in_map entries, inputs)."""
    ins = {k: np.ascontiguousarray(np.asarray(v, dtype=np.float32))
           for k, v in inputs.items()}
    simW = ins["simW"][:, 0]
    simb = float(ins["simb"][0])
    cg = ins["cg"]
    cbeta = ins["cbeta"]
    absw = np.abs(simW)
    sgn = np.sign(simW).astype(np.float32)
    if not np.allclose(cbeta * absw, 0.0):
        raise NotImplementedError(
            "kernel specialized for cbeta*|simW|==0 (true for this problem)")
    gvec = (cg * absw).astype(np.float32)
    gvec2 = np.concatenate([gvec, gvec])
    sgn2 = np.zeros((128, 2), np.float32)
    sgn2[0:64, 0] = sgn
    sgn2[64:128, 1] = sgn
    sgng = (cg * simW).astype(np.float32)

    trivial = {
        "sb1": np.allclose(ins["sb1"], 0), "sg": np.allclose(ins["sg"], 1),
        "sbeta": np.allclose(ins["sbeta"], 0), "sb2": np.allclose(ins["sb2"], 0),
        "tb1": np.allclose(ins["tb1"], 0), "tg": np.allclose(ins["tg"], 1),
        "tbeta": np.allclose(ins["tbeta"], 0), "tb2": np.allclose(ins["tb2"], 0),
        "cb": np.allclose(ins["cb"], 0),
    }
    wmeta = {"simb": simb, "trivial": trivial}

    common = {
        "sW1": ins["sW1"], "sW2": ins["sW2"], "tW1": ins["tW1"],
        "tW2": ins["tW2"], "cWt": ins["cW"][:H], "cWs": ins["cW"][H:],
        "gvec2": gvec2, "sgn2": sgn2, "sgng": sgng,
    }
    for name in ("sb1", "sg", "sbeta", "sb2", "tb1", "tg", "tbeta", "tb2", "cb"):
        if not trivial[name]:
            common[name] = ins[name]
    return wmeta, common, ins


def _in_maps(common, ins):
    maps = []
    for core in range(NCORES):
        b, q4 = core // 4, core % 4
        m = dict(common)
        m["src"] = np.ascontiguousarray(ins["source_features"][b])
        m["tgt"] = np.ascontiguousarray(
            ins["target_features"][b, q4 * 256:(q4 + 1) * 256])
        maps.append(m)
    return maps


def kernel(**inputs):
    from concourse.bass_utils import run_bass_kernel_spmd

    wmeta, common, ins = _prep(inputs)
    if "prog" not in _CACHE:
        _CACHE["prog"] = _build_program(wmeta)
    nc = _CACHE["prog"]

    res = run_bass_kernel_spmd(nc, _in_maps(common, ins), list(range(NCORES)))
    adapted = np.zeros((B, T, H), np.float32)
    scores = np.zeros((B, T, S), np.float32)
    for core in range(NCORES):
        b, q4 = core // 4, core % 4
        r = res.results[core]
        adapted[b, q4 * 256:(q4 + 1) * 256] = r["adapted_out"]
        scores[b, q4 * 256:(q4 + 1) * 256] = r["scores_out"]
    return adapted, scores


# revision 38
# speedup vs baseline: 1.0220x; 1.0220x over previous
"""CrossCityAdapter Trainium2 kernel (8 NeuronCores, SPMD).

Math (per batch b):
  s_ad = relu(LN(src @ sW1 + sb1)) @ sW2 + sb2          [S,H]
  t_ad = relu(LN(tgt @ tW1 + tb1)) @ tW2 + tb2          [T,H]
  pre[t,s,:] = t_ad@cW_t[t,:] + (s_ad@cW_s + cb)[s,:]   (separable over t/s)
  h = relu(LN(pre)*cg + cbeta);  scores = sigmoid(h @ simW + simb)
  attn = softmax(scores, -1); transferred = attn @ s_ad
  gate = sigmoid(mean(t_ad)); adapted = t_ad*(1-gate) + transferred*gate

Key identities exploited on-device:
  * LN stats of pre decompose: mean = mt[t]+ms[s],
    var = a[t] + c[s] + (2/H) * (xhat @ vhat^T)[t,s]  -> one PE matmul
    (a/c/1-rows appended to the operands so one matmul yields full var).
  * With cbeta*|simW| == 0 (true for these inputs) and w = rstd > 0:
      sum_h simW[h]*relu((xhat+vhat)*w*cg + cbeta)
        = w * (sum_h sgn[h]*max(xtil,-vtil) + K[s]),   xtil = xhat*cg*|simW|
    using relu(a+b) = max(a,-b)+b and K[s] = sum_h vhat[s,h]*cg[h]*simW[h].
    So the only O(T*S*H) elementwise op is one bf16 max on DVE. The signed
    H-reduction runs on the tensor engine: the max-result r (h-stacked for
    two targets, K=128 = 2x64) is the STATIONARY operand and the +-1 sign
    matrix streams, so PSUM gets [128 s, 2] full-partition outputs (exact:
    bf16 * {+-1,0} accumulated in fp32).
  * Everything downstream of the sign-reduce is s-major; scores are
    PE-transposed back to t-major only for the final DMA, and the s-major
    exp(scores) feeds the attn matmul as lhsT with no transpose at all.
  * sigmoid outputs lie in (0,1) so softmax needs no max-subtraction.

Sharding: 8 cores = batch (2) x T-quarters (4); each core owns 256 targets,
all S. No collectives; host gathers core outputs.

Target pairing: column c in [0,128) pairs local targets (c, c+128);
"slot" 2c+j (j = half) <-> t = c + 128j. PSUM/num rows use slot order;
DRAM APs unscramble on the way out.
"""

import numpy as np

B, S, T, DS, DT, H = 2, 1024, 1024, 128, 64, 64
LN_EPS = 1e-5
NCORES = 8
TLOC = T * B // NCORES  # 256 target rows per core

_CACHE = {}


def _build_program(wmeta):
    import concourse.bacc as bacc
    import concourse.bass as bass
    import concourse.tile as tile
    from concourse import mybir
    from concourse.masks import make_identity

    f32 = mybir.dt.float32
    bf16 = mybir.dt.bfloat16
    AF = mybir.ActivationFunctionType
    OP = mybir.AluOpType
    AX = mybir.AxisListType

    simb = wmeta["simb"]
    triv = wmeta["trivial"]

    nc = bacc.Bacc("TRN2", target_bir_lowering=False, debug=False)

    # ---- DRAM I/O ------------------------------------------------------
    src = nc.declare_dram_parameter("src", [S, DS], f32, isOutput=False).ap()
    tgt = nc.declare_dram_parameter("tgt", [TLOC, DT], f32, isOutput=False).ap()
    sW1 = nc.declare_dram_parameter("sW1", [DS, H], f32, isOutput=False).ap()
    sW2 = nc.declare_dram_parameter("sW2", [H, H], f32, isOutput=False).ap()
    tW1 = nc.declare_dram_parameter("tW1", [DT, H], f32, isOutput=False).ap()
    tW2 = nc.declare_dram_parameter("tW2", [H, H], f32, isOutput=False).ap()
    cWt = nc.declare_dram_parameter("cWt", [H, H], f32, isOutput=False).ap()
    cWs = nc.declare_dram_parameter("cWs", [H, H], f32, isOutput=False).ap()
    gvec2 = nc.declare_dram_parameter("gvec2", [128], f32, isOutput=False).ap()
    sgn2 = nc.declare_dram_parameter("sgn2", [128, 2], f32, isOutput=False).ap()
    sgng = nc.declare_dram_parameter("sgng", [H], f32, isOutput=False).ap()
    nontriv_vecs = {}
    for name in ("sb1", "sg", "sbeta", "sb2", "tb1", "tg", "tbeta", "tb2", "cb"):
        if not triv[name]:
            nontriv_vecs[name] = nc.declare_dram_parameter(
                name, [H], f32, isOutput=False
            ).ap()
    scores_out = nc.declare_dram_parameter(
        "scores_out", [TLOC, S], f32, isOutput=True
    ).ap()
    adapted_out = nc.declare_dram_parameter(
        "adapted_out", [TLOC, H], f32, isOutput=True
    ).ap()

    def bcast_row(ap, n):
        return bass.AP(tensor=ap.tensor, offset=ap.offset,
                       ap=[[0, 128], [1, n]])

    _ps_pools = []
    _ps_ctr = [0]
    with tile.TileContext(nc) as tc:
        with (
            tc.tile_pool(name="singles", bufs=1) as singles,
            tc.tile_pool(name="weights", bufs=1) as wpool,
            tc.tile_pool(name="temps", bufs=4) as temps,
            tc.tile_pool(name="cols", bufs=4) as cols,
            tc.tile_pool(name="rbig", bufs=6) as rbig_pool,
            tc.tile_pool(name="smaj", bufs=4) as smaj,
            # PSUM budget (8 banks): psT 3 + psSC 3 + psN 2 = 8
            tc.tile_pool(name="psT", bufs=3, space="PSUM") as psT,
            tc.tile_pool(name="psSC", bufs=3, space="PSUM") as psSC,
            tc.tile_pool(name="psN", bufs=2, space="PSUM") as psN,
            tc.tile_pool(name="dram", bufs=1, space="DRAM") as dram,
        ):
            _ps_pools.extend([(psT, "ps")])
            def ps128():
                """Round-robin a [128,128] psum tile across all pools
                (psSC/psN are idle during the adapter phase)."""
                pool, tag = _ps_pools[_ps_ctr[0] % len(_ps_pools)]
                _ps_ctr[0] += 1
                return pool.tile([128, 128], f32, tag=tag, name="ps128")

            # ---- constants / weights in SBUF --------------------------
            ident = singles.tile([128, 128], f32)
            make_identity(nc, ident[:])

            sW1_sb = wpool.tile([DS, H], f32)
            nc.sync.dma_start(out=sW1_sb[:], in_=sW1)
            w64 = {}
            for name, ap in (("sW2", sW2), ("tW1", tW1), ("tW2", tW2),
                             ("cWt", cWt), ("cWs", cWs)):
                t_ = wpool.tile([H, H], f32, tag=name, name=f"w_{name}")
                nc.sync.dma_start(out=t_[:], in_=ap)
                w64[name] = t_

            gvec2_col = wpool.tile([128, 1], f32)
            nc.sync.dma_start(out=gvec2_col[:], in_=gvec2[:, None])

            sgn2_f = wpool.tile([128, 2], f32)
            nc.sync.dma_start(out=sgn2_f[:], in_=sgn2)
            sgn2_bf = wpool.tile([128, 2], bf16)
            nc.vector.tensor_copy(sgn2_bf[:], sgn2_f[:])

            sgng_rep = wpool.tile([128, H], f32)
            nc.sync.dma_start(out=sgng_rep[:], in_=bcast_row(sgng, H))

            eps_col = wpool.tile([128, 1], f32)
            nc.vector.memset(eps_col[:], LN_EPS)

            vrep = {}
            for name, ap in nontriv_vecs.items():
                t_ = wpool.tile([128, H], f32, tag=f"{name}_rep",
                                name=f"rep_{name}")
                nc.sync.dma_start(out=t_[:], in_=bcast_row(ap, H))
                vrep[name] = t_

            # ---- persistent device tensors ----------------------------
            # (split by s-block so the main loop overlaps the adapters)
            xstk = singles.tile([128, 128], f32)      # rows h|h: xtil^T t|t+128
            vstk = [singles.tile([128, 512], bf16, name=f"vstk{z}")
                    for z in range(2)]                # rows h|h: -vtil^T (dup)
            xhatT_ext = singles.tile([66, TLOC], f32)  # [xhat^T; ones; a+eps]
            vext = [singles.tile([66, 128], f32, name=f"vext{u}")
                    for u in range(S // 128)]          # [vhat^T*2/H; c; 1]
            sad_t = [singles.tile([128, H + 1], f32, name=f"sad{u}")
                     for u in range(S // 128)]
            t_ad_sb = singles.tile([128, 2, H], f32)
            kcol_t = [singles.tile([128, 1], f32, name=f"kcol{u}")
                      for u in range(S // 128)]        # K[s], s-major

            a_dram = dram.tile([TLOC], f32)
            c_dram = [dram.tile([128], f32, name=f"c_dram{u}")
                      for u in range(S // 128)]
            tad_dram = dram.tile([TLOC, H], f32)
            gate_dram = dram.tile([TLOC], f32)

            nc.vector.memset(xhatT_ext[64:66, :], 1.0)


            def ln_relu(a_ps, bias_rep, g_rep, beta_rep):
                """psum [128,H] -> relu(LN(.)) in sbuf."""
                a_sb_t = temps.tile([128, H], f32, tag="ln_in",
                                    name="ln_in")
                if bias_rep is not None:
                    nc.vector.tensor_add(a_sb_t[:], a_ps, bias_rep[:])
                else:
                    nc.scalar.copy(a_sb_t[:], a_ps)
                a_sb = a_sb_t[:]
                st6 = temps.tile([128, 6], f32, tag="ln_st", name="ln_st")
                mv = temps.tile([128, 2], f32, tag="ln_mv", name="ln_mv")
                nc.vector.bn_stats(out=st6[:], in_=a_sb)
                nc.vector.bn_aggr(out=mv[:], in_=st6[:])
                std = temps.tile([128, 1], f32, tag="ln_std", name="ln_std")
                nc.scalar.activation(std[:], mv[:, 1:2], AF.Sqrt,
                                     bias=eps_col[:], scale=1.0)
                rstd = temps.tile([128, 1], f32, tag="ln_rstd", name="ln_rstd")
                nc.vector.reciprocal(rstd[:], std[:])
                xn = temps.tile([128, H], f32, tag="ln_xn", name="ln_xn")
                nc.vector.tensor_scalar(
                    out=xn[:], in0=a_sb, scalar1=mv[:, 0:1],
                    scalar2=rstd[:], op0=OP.subtract, op1=OP.mult)
                if g_rep is not None:
                    nc.vector.tensor_mul(xn[:], xn[:], g_rep[:])
                if beta_rep is not None:
                    nc.vector.tensor_add(xn[:], xn[:], beta_rep[:])
                out = temps.tile([128, H], f32, tag="ln_out", name="ln_out")
                nc.scalar.activation(out[:], xn[:], AF.Relu)
                return out

            def transpose_small(in_ap, tag="tp"):
                """[128,F<=128] sbuf -> [F,128] sbuf via PE transpose."""
                ps = ps128()
                fsz = in_ap.shape[-1]
                nc.tensor.transpose(ps[0:fsz, :], in_ap, ident[:])
                sb = temps.tile([128, 128], f32, tag=tag, name=f"sb_{tag}")
                nc.scalar.copy(sb[0:fsz, :], ps[0:fsz, :])
                return sb

            # ---- target adapter (256 rows) ----------------------------
            gate_ps = []
            xstk_stage = temps.tile([64, 128], f32, tag="xstg",
                                    name="xstk_stage")
            for j in range(2):
                x_sb = temps.tile([128, DT], f32, tag="tgt_in", name="tgt_in")
                nc.sync.dma_start(out=x_sb[:], in_=tgt[j * 128:(j + 1) * 128, :])
                xT_sb = transpose_small(x_sb[:], tag="tgtT")
                a1_ps = ps128()
                nc.tensor.matmul(a1_ps[:, 0:H], xT_sb[0:DT, :], w64["tW1"][:])
                a1r = ln_relu(a1_ps[:, 0:H], vrep.get("tb1"), vrep.get("tg"),
                              vrep.get("tbeta"))
                a1rT = transpose_small(a1r[:], tag="a1rT")
                tad_ps = ps128()
                nc.tensor.matmul(tad_ps[:, 0:H], a1rT[0:H, :], w64["tW2"][:])
                if "tb2" in vrep:
                    nc.vector.tensor_add(t_ad_sb[:, j, :], tad_ps[:, 0:H],
                                         vrep["tb2"][:])
                else:
                    nc.scalar.copy(t_ad_sb[:, j, :], tad_ps[:, 0:H])
                nc.sync.dma_start(out=tad_dram[j * 128:(j + 1) * 128, :],
                                  in_=t_ad_sb[:, j, :])
                gsum = cols.tile([128, 1], f32, tag=f"gps{j}", name=f"gps{j}")
                nc.vector.tensor_reduce(out=gsum[:], in_=t_ad_sb[:, j, :],
                                        axis=AX.X, op=OP.add)
                nc.sync.dma_start(out=gate_dram[j * 128:(j + 1) * 128],
                                  in_=gsum[:])
                gate_ps.append(gsum)

                tadT = transpose_small(t_ad_sb[:, j, :], tag="tadT")
                xp_ps = ps128()
                nc.tensor.matmul(xp_ps[:, 0:H], tadT[0:H, :], w64["cWt"][:])
                x_pre_t = temps.tile([128, H], f32, tag="x_pre",
                                     name="x_pre")
                nc.scalar.copy(x_pre_t[:], xp_ps[:, 0:H])
                x_pre = x_pre_t[:]
                st6 = temps.tile([128, 6], f32, tag="ln_st", name="st6t")
                mv = temps.tile([128, 2], f32, tag="ln_mv", name="mvt")
                nc.vector.bn_stats(out=st6[:], in_=x_pre)
                nc.vector.bn_aggr(out=mv[:], in_=st6[:])
                a_eps = temps.tile([128, 1], f32, tag="a_eps", name="a_eps")
                nc.vector.tensor_scalar(out=a_eps[:], in0=mv[:, 1:2],
                                        scalar1=LN_EPS, scalar2=None,
                                        op0=OP.add)
                nc.sync.dma_start(out=a_dram[j * 128:(j + 1) * 128],
                                  in_=a_eps[:])
                xhat = temps.tile([128, H], f32, tag="xhat", name="xhat")
                nc.vector.tensor_scalar(out=xhat[:], in0=x_pre,
                                        scalar1=mv[:, 0:1], scalar2=None,
                                        op0=OP.subtract)
                xf_ps = ps128()
                nc.tensor.transpose(xf_ps[0:H, :], xhat[:], ident[:])
                nc.scalar.copy(xhatT_ext[0:H, j * 128:(j + 1) * 128],
                               xf_ps[0:H, :])
                if j == 0:
                    nc.vector.tensor_scalar(
                        out=xstk[0:H, :], in0=xf_ps[0:H, :],
                        scalar1=gvec2_col[0:H, :], scalar2=None, op0=OP.mult)
                else:
                    nc.vector.tensor_scalar(
                        out=xstk_stage[:], in0=xf_ps[0:H, :],
                        scalar1=gvec2_col[0:H, :], scalar2=None, op0=OP.mult)
                    nc.sync.dma_start(out=xstk[H:128, :], in_=xstk_stage[:])
            nc.sync.dma_start(out=xhatT_ext[64:65, :],
                              in_=bass.AP(tensor=a_dram[:].tensor,
                                          offset=a_dram[:].offset,
                                          ap=[[0, 1], [1, TLOC]]))

            # ---- source adapter + v-side (full S) ---------------------
            for i in range(S // 128):
                x_sb = temps.tile([128, DS], f32, tag="src_in", name="src_in")
                nc.sync.dma_start(out=x_sb[:], in_=src[i * 128:(i + 1) * 128, :])
                xT_sb = transpose_small(x_sb[:], tag="srcT")
                a1_ps = ps128()
                nc.tensor.matmul(a1_ps[:, 0:H], xT_sb[:], sW1_sb[:])
                a1r = ln_relu(a1_ps[:, 0:H], vrep.get("sb1"), vrep.get("sg"),
                              vrep.get("sbeta"))
                a1rT = transpose_small(a1r[:], tag="a1rT")
                sad_ps = ps128()
                nc.tensor.matmul(sad_ps[:, 0:H], a1rT[0:H, :], w64["sW2"][:])
                if "sb2" in vrep:
                    nc.vector.tensor_add(sad_t[i][:, 0:H], sad_ps[:, 0:H],
                                         vrep["sb2"][:])
                else:
                    nc.scalar.copy(sad_t[i][:, 0:H], sad_ps[:, 0:H])
                nc.vector.memset(sad_t[i][:, H:H + 1], 1.0)

                sadT = transpose_small(sad_t[i][:, 0:H], tag="sadT")
                v_ps = ps128()
                nc.tensor.matmul(v_ps[:, 0:H], sadT[0:H, :], w64["cWs"][:])
                v_pre_t = temps.tile([128, H], f32, tag="v_pre",
                                     name="v_pre")
                if "cb" in vrep:
                    nc.vector.tensor_add(v_pre_t[:], v_ps[:, 0:H],
                                         vrep["cb"][:])
                else:
                    nc.scalar.copy(v_pre_t[:], v_ps[:, 0:H])
                v_pre = v_pre_t[:]
                st6 = temps.tile([128, 6], f32, tag="ln_st", name="st6v")
                mv = temps.tile([128, 2], f32, tag="ln_mv", name="mvv")
                nc.vector.bn_stats(out=st6[:], in_=v_pre)
                nc.vector.bn_aggr(out=mv[:], in_=st6[:])
                vhat = temps.tile([128, H], f32, tag="vhat", name="vhat")
                nc.vector.tensor_scalar(out=vhat[:], in0=v_pre,
                                        scalar1=mv[:, 0:1], scalar2=None,
                                        op0=OP.subtract)
                # K[s] = sum_h vhat*cg*simW  (stays per-partition, s-major)
                kv = temps.tile([128, H], f32, tag="kv", name="kv")
                nc.vector.tensor_mul(kv[:], vhat[:], sgng_rep[:])
                nc.vector.tensor_reduce(out=kcol_t[i][:], in_=kv[:],
                                        axis=AX.X, op=OP.add)
                vT_ps = ps128()
                nc.tensor.transpose(vT_ps[0:H, :], vhat[:], ident[:])
                nc.vector.tensor_scalar(
                    out=vext[i][0:H, :],
                    in0=vT_ps[0:H, :], scalar1=2.0 / H, scalar2=None,
                    op0=OP.mult)
                zz, zo = i // 4, (i % 4) * 128
                nc.vector.tensor_scalar(
                    out=vstk[zz][0:H, zo:zo + 128], in0=vT_ps[0:H, :],
                    scalar1=gvec2_col[0:H, :], scalar2=-1.0,
                    op0=OP.mult, op1=OP.mult)
                nc.sync.dma_start(out=vstk[zz][H:128, zo:zo + 128],
                                  in_=vstk[zz][0:H, zo:zo + 128])
                nc.sync.dma_start(out=c_dram[i][:], in_=mv[:, 1:2])
                # rows 64:66 <- [ones; c]: broadcast c into both (base-64
                # DMA), then memset row 64 back to 1.0 (base-64 DVE op).
                nc.sync.dma_start(
                    out=vext[i][64:66, :],
                    in_=bass.AP(tensor=c_dram[i][:].tensor,
                                offset=c_dram[i][:].offset,
                                ap=[[0, 2], [1, 128]]))
                nc.vector.memset(vext[i][64:65, :], 1.0)

            # funny-gathered epilogue inputs per half f:
            #   row r <-> t = 64*f + r//2 + 128*(r&1)
            tad_re = tad_dram[:].rearrange("(j c) h -> c j h", j=2)
            gate_re = gate_dram[:].rearrange("(j c) -> c j", j=2)
            tadf, gatef = [], []
            for f in range(2):
                tf = cols.tile([128, H], f32, tag="tadf", name=f"tadf{f}")
                nc.sync.dma_start(out=tf[:],
                                  in_=tad_re[64 * f:64 * (f + 1), :, :])
                tadf.append(tf)
                gf = cols.tile([128, 1], f32, tag="gatef", name=f"gatef{f}")
                nc.sync.dma_start(out=gf[:],
                                  in_=gate_re[64 * f:64 * (f + 1), :])
                gatef.append(gf)

            # ---- main pipeline, phase-ordered to batch ACT table sets --
            # xhatT funny view: col (2c+j) <-> t = c + 128j
            xfun = xhatT_ext[:].rearrange("p (j c) -> p c j", j=2)
            scout_re = scores_out.rearrange("(j c) s -> c j s", j=2)
            num_ps = [psN.tile([128, H + 1], f32, tag="num", name=f"num{f}")
                      for f in range(2)]
            NB = S // 128  # 8 s-blocks

            # W-phase: w = rsqrt(var) per s-block (Sqrt set stays loaded)
            w_t = []
            sqrt_insts = []
            for i_s in range(NB):
                var_ps = psT.tile([128, 256], f32, tag="ps", name="var_ps")
                nc.tensor.matmul(var_ps[:], vext[i_s][:], xfun)
                std = temps.tile([128, 256], f32, tag="std", name="std")
                sqrt_insts.append(
                    nc.scalar.activation(std[:], var_ps[:], AF.Sqrt))
                w_ = smaj.tile([128, 256], f32, tag="w", name=f"w{i_s}",
                               bufs=8)
                nc.vector.reciprocal(w_[:], std[:])
                w_t.append(w_)

            # R-phase: bf16 max (tensor_scalar 4x) + PE sign-reduce + STT
            scorep = [None] * NB
            for sh in range(4):  # half of an s-512 chunk: 2 s-blocks
                sc, uh = sh // 2, sh % 2
                ps_u = [psSC.tile([128, 256], f32, tag="sc", name=f"ps_sc{u}")
                        for u in range(2)]
                for cg in range(16):
                    r_ = rbig_pool.tile([128, 8, 256], bf16, tag="rbig",
                                        name="rbig")
                    for cl in range(8):
                        nc.vector.tensor_scalar_max(
                            r_[:, cl, :],
                            vstk[sc][:, 256 * uh:256 * (uh + 1)],
                            xstk[:, 8 * cg + cl:8 * cg + cl + 1])
                    for u in range(2):
                        for cl in range(8):
                            c = 8 * cg + cl
                            nc.tensor.matmul(
                                ps_u[u][:, 2 * c:2 * c + 2],
                                r_[:, cl, 128 * u:128 * (u + 1)],
                                sgn2_bf[:], start=True, stop=True)
                for u in range(2):
                    i_s = 2 * sh + u
                    sp = smaj.tile([128, 256], f32, tag="scorep",
                                   name=f"scorep{i_s}", bufs=8)
                    nc.vector.scalar_tensor_tensor(
                        out=sp[:], in0=ps_u[u][:],
                        scalar=kcol_t[i_s][:], in1=w_t[i_s][:],
                        op0=OP.add, op1=OP.mult)
                    scorep[i_s] = sp

            # SIG-phase (one sigmoid table load)
            from concourse.tile import add_dep_helper
            gsig_t = []
            sig_insts = []
            for f in range(2):
                g_ = cols.tile([128, 1], f32, tag="gsig", name=f"gsig{f}")
                gm_ = cols.tile([128, 1], f32, tag="gm", name=f"gm{f}")
                nc.vector.tensor_scalar(out=gm_[:], in0=gatef[f][:],
                                        scalar1=1.0 / H, scalar2=None,
                                        op0=OP.mult)
                si = nc.scalar.activation(g_[:], gm_[:], AF.Sigmoid)
                add_dep_helper(si.ins, sqrt_insts[-1].ins, sync=False,
                               reason="batch ACT table sets")
                sig_insts.append(si)
                gsig_t.append(g_)
            scores_t = []
            for i_s in range(NB):
                ssb = smaj.tile([128, 256], f32, tag="scores",
                                name=f"ssb{i_s}", bufs=8)
                si = nc.scalar.activation(ssb[:], scorep[i_s][:], AF.Sigmoid,
                                          bias=float(simb), scale=1.0)
                add_dep_helper(si.ins, sqrt_insts[-1].ins, sync=False,
                               reason="batch ACT table sets")
                sig_insts.append(si)
                scores_t.append(ssb)
            exp_deps = {}
            # transpose halves to t-major, stage, and DMA in 2 big bursts
            sc_stage = [singles.tile([128, S], f32, name=f"sc_stage{f}")
                        for f in range(2)]
            for i_s in range(NB):
                for f in range(2):
                    sT_ps = psT.tile([128, 128], f32, tag="ps",
                                     name="sT_ps")
                    nc.tensor.transpose(sT_ps[:],
                                        scores_t[i_s][:, 128 * f:128 * (f + 1)],
                                        ident[:])
                    nc.vector.tensor_copy(
                        sc_stage[f][:, 128 * i_s:128 * (i_s + 1)], sT_ps[:])
            for f in range(2):
                nc.sync.dma_start(out=scout_re[64 * f:64 * (f + 1), :, :],
                                in_=sc_stage[f][:])

            # EXP-phase (one exp table load) + attn matmuls
            for i_s in range(NB):
                e_ = smaj.tile([128, 256], f32, tag="E", name=f"e{i_s}",
                               bufs=4)
                ei = nc.scalar.activation(e_[:], scores_t[i_s][:], AF.Exp)
                dep = sig_insts[2 + 5] if i_s < 6 else sig_insts[-1]
                add_dep_helper(ei.ins, dep.ins, sync=False,
                               reason="batch ACT table sets")
                if i_s == 5:
                    exp_deps["last_early"] = ei
                    # group sig6/sig7 after the early exps (4 loads total)
                    add_dep_helper(sig_insts[2 + 6].ins, ei.ins, sync=False,
                                   reason="batch ACT table sets")
                    add_dep_helper(sig_insts[2 + 7].ins, ei.ins, sync=False,
                                   reason="batch ACT table sets")
                for f in range(2):
                    nc.tensor.matmul(num_ps[f][:],
                                     e_[:, 128 * f:128 * (f + 1)],
                                     sad_t[i_s][:],
                                     start=(i_s == 0), stop=(i_s == NB - 1),
                                     skip_group_check=True)

            # ---- epilogue: adapted (slot-row order, unscrambled by DMA)
            adout_re = adapted_out.rearrange("(j c) h -> c j h", j=2)
            for f in range(2):
                zrec = cols.tile([128, 1], f32, tag="zrec", name=f"zrec{f}")
                nc.vector.reciprocal(zrec[:], num_ps[f][:, H:H + 1])
                gsig = gsig_t[f]
                trans = cols.tile([128, H], f32, tag="trans", name=f"tr{f}")
                nc.vector.tensor_scalar(out=trans[:], in0=num_ps[f][:, 0:H],
                                        scalar1=zrec[:], scalar2=None,
                                        op0=OP.mult)
                d_ = cols.tile([128, H], f32, tag="dtile", name=f"d{f}")
                nc.vector.tensor_sub(d_[:], trans[:], tadf[f][:])
                ad_ = cols.tile([128, H], f32, tag="adf", name=f"ad{f}")
                nc.vector.scalar_tensor_tensor(
                    out=ad_[:], in0=d_[:], scalar=gsig[:],
                    in1=tadf[f][:], op0=OP.mult, op1=OP.add)
                nc.sync.dma_start(out=adout_re[64 * f:64 * (f + 1), :, :],
                                  in_=ad_[:])

    nc.compile()
    return nc


def _prep(inputs):
    """Host-side weight prep. Returns (wmeta, common in_map entries, inputs)."""
    ins = {k: np.ascontiguousarray(np.asarray(v, dtype=np.float32))
           for k, v in inputs.items()}
    simW = ins["simW"][:, 0]
    simb = float(ins["simb"][0])
    cg = ins["cg"]
    cbeta = ins["cbeta"]
    absw = np.abs(simW)
    sgn = np.sign(simW).astype(np.float32)
    if not np.allclose(cbeta * absw, 0.0):
        raise NotImplementedError(
            "kernel specialized for cbeta*|simW|==0 (true for this problem)")
    gvec = (cg * absw).astype(np.float32)
    gvec2 = np.concatenate([gvec, gvec])
    sgn2 = np.zeros((128, 2), np.float32)
    sgn2[0:64, 0] = sgn
    sgn2[64:128, 1] = sgn
    sgng = (cg * simW).astype(np.float32)

    trivial = {
        "sb1": np.allclose(ins["sb1"], 0), "sg": np.allclose(ins["sg"], 1),
        "sbeta": np.allclose(ins["sbeta"], 0), "sb2": np.allclose(ins["sb2"], 0),
        "tb1": np.allclose(ins["tb1"], 0), "tg": np.allclose(ins["tg"], 1),
        "tbeta": np.allclose(ins["tbeta"], 0), "tb2": np.allclose(ins["tb2"], 0),
        "cb": np.allclose(ins["cb"], 0),
    }
    wmeta = {"simb": simb, "trivial": trivial}

    common = {
        "sW1": ins["sW1"], "sW2": ins["sW2"], "tW1": ins["tW1"],
        "tW2": ins["tW2"], "cWt": ins["cW"][:H], "cWs": ins["cW"][H:],
        "gvec2": gvec2, "sgn2": sgn2, "sgng": sgng,
    }
    for name in ("sb1", "sg", "sbeta", "sb2", "tb1", "tg", "tbeta", "tb2", "cb"):
        if not trivial[name]:
            common[name] = ins[name]
    return wmeta, common, ins


def _in_maps(common, ins):
    maps = []
    for core in range(NCORES):
        b, q4 = core // 4, core % 4
        m = dict(common)
        m["src"] = np.ascontiguousarray(ins["source_features"][b])
        m["tgt"] = np.ascontiguousarray(
            ins["target_features"][b, q4 * 256:(q4 + 1) * 256])
        maps.append(m)
    return maps


def kernel(**inputs):
    from concourse.bass_utils import run_bass_kernel_spmd

    wmeta, common, ins = _prep(inputs)
    if "prog" not in _CACHE:
        _CACHE["prog"] = _build_program(wmeta)
    nc = _CACHE["prog"]

    res = run_bass_kernel_spmd(nc, _in_maps(common, ins), list(range(NCORES)))
    adapted = np.zeros((B, T, H), np.float32)
    scores = np.zeros((B, T, S), np.float32)
    for core in range(NCORES):
        b, q4 = core // 4, core % 4
        r = res.results[core]
        adapted[b, q4 * 256:(q4 + 1) * 256] = r["adapted_out"]
        scores[b, q4 * 256:(q4 + 1) * 256] = r["scores_out"]
    return adapted, scores
